# revision 1
# baseline (speedup 1.0000x reference)
"""nn_ColorReducer — Trainium2 Bass kernel (8-core data-parallel).

Algorithm (exact nearest-palette-color, argmax-free):
  score_k(p) = 2 p·c_k - ||c_k||^2  (maximizing k <=> nearest color)
  Scores computed on TensorE with pixels as the stationary operand and the
  palette (bf16 3-limb split for ~f32 accuracy) as the moving operand, so
  scores land in PSUM as (128 pixels, 64 colors) — reducible along the free
  dim. VectorE takes a grouped reduce-max, subtracts it exactly (s - max = 0
  bitwise at the winner), ScalarE applies w = exp(beta*(s-max)) which
  annihilates non-winners (beta = 2^22), and VectorE contracts w against the
  palette channels for the output color + a weight-count for host-side
  normalization of rounding-level ties.

Sharding: batch dim (8 images) across the 8 NeuronCores; palette replicated.
"""

import numpy as np
import ml_dtypes

bf16 = ml_dtypes.bfloat16

B, C, H, W = 8, 3, 512, 512
HW = H * W              # 262144 pixels per core
NCOL = 64               # palette entries
KROWS = 21              # limb product rows: 6 per channel + 3 bias rows
PXT = 128               # pixels per matmul tile (stationary columns)
TPG = 8                 # matmul tiles per PSUM-bank group
GPX = PXT * TPG         # 1024 pixels per group
NTILES = HW // PXT      # 2048
GROUPS = NTILES // TPG  # 256
BETA = float(2 ** 22)

_CACHE = {}


def _split3(x):
    """f32 -> three bf16 limbs (h, m, l) with h+m+l ~ x to ~2^-24 rel."""
    x = np.asarray(x, np.float32)
    h = x.astype(bf16)
    r = x - h.astype(np.float32)
    m = r.astype(bf16)
    r2 = r - m.astype(np.float32)
    l = r2.astype(bf16)
    return h, m, l


def _host_pixel_limbs(px):
    """px: (3, HW) f32 -> (KROWS, HW) bf16 stationary rows.

    Per channel the 6 product rows pair pixel limbs with palette limbs:
      (ph,wh) (ph,wm) (pm,wh) (pm,wm) (ph,wl) (pl,wh)
    so the pixel side is [ph, ph, pm, pm, ph, pl]; rows 18-20 are ones
    (paired with the 3 bias limbs).
    """
    rows = np.empty((KROWS, px.shape[1]), dtype=bf16)
    for c in range(3):
        h, m, l = _split3(px[c])
        rows[6 * c + 0] = h
        rows[6 * c + 1] = h
        rows[6 * c + 2] = m
        rows[6 * c + 3] = m
        rows[6 * c + 4] = h
        rows[6 * c + 5] = l
    rows[18:21] = np.ones((3, px.shape[1]), dtype=bf16)
    return rows


def _host_palette_gather(palette):
    """(64,3) f32 -> (128, 14) bf16 block-diag gather lhsT.

    Rows 0-63 (colors of even px-tile) -> cols 0-6 = [rh,rl,gh,gl,bh,bl,1];
    rows 64-127 (odd px-tile) -> cols 7-13. Channel values as 2 bf16 limbs
    summed on the host.
    """
    pal = np.asarray(palette, np.float32)
    g = np.zeros((128, 14), dtype=bf16)
    cols = np.empty((NCOL, 7), np.float32)
    for c in range(3):
        h = pal[:, c].astype(bf16)
        l = (pal[:, c] - h.astype(np.float32)).astype(bf16)
        cols[:, 2 * c] = h.astype(np.float32)
        cols[:, 2 * c + 1] = l.astype(np.float32)
    cols[:, 6] = 1.0
    g[0:NCOL, 0:7] = cols.astype(bf16)
    g[NCOL:128, 7:14] = cols.astype(bf16)
    return g


def _host_palette_rows(palette):
    """palette: (64, 3) f32 -> (KROWS, 64) bf16 moving-operand rows."""
    pal = np.asarray(palette, np.float64)
    rows = np.empty((KROWS, NCOL), dtype=bf16)
    for c in range(3):
        wh, wm, wl = _split3((2.0 * pal[:, c]).astype(np.float32))
        rows[6 * c + 0] = wh
        rows[6 * c + 1] = wm
        rows[6 * c + 2] = wh
        rows[6 * c + 3] = wm
        rows[6 * c + 4] = wl
        rows[6 * c + 5] = wh
    b64 = -(pal ** 2).sum(axis=1)
    bh = b64.astype(np.float32).astype(bf16)
    r = b64 - bh.astype(np.float64)
    bm = r.astype(np.float32).astype(bf16)
    r2 = r - bm.astype(np.float64)
    bl = r2.astype(np.float32).astype(bf16)
    rows[18], rows[19], rows[20] = bh, bm, bl
    return rows


def _build_body(nc, tc, ctx, aps, n_tiles):
    """Emit the per-core program. aps: dict name->AP for dram tensors."""
    import concourse.bass as bass
    import concourse.mybir as mybir

    n_groups = n_tiles // TPG
    f32 = mybir.dt.float32
    bft = mybir.dt.bfloat16

    consts = ctx.enter_context(tc.tile_pool(name="consts", bufs=1))
    limb_pool = ctx.enter_context(tc.tile_pool(name="limb", bufs=3))
    psum_pool = ctx.enter_context(tc.tile_pool(name="psum", bufs=2, space="PSUM"))
    gps_pool = ctx.enter_context(tc.tile_pool(name="gps", bufs=2, space="PSUM"))
    sadj_pool = ctx.enter_context(tc.tile_pool(name="sadj", bufs=2))
    w_pool = ctx.enter_context(tc.tile_pool(name="w", bufs=2))
    wt_pool = ctx.enter_context(tc.tile_pool(name="wt", bufs=2))
    small_pool = ctx.enter_context(tc.tile_pool(name="small", bufs=3))

    palT = consts.tile([KROWS, NCOL], bft)
    nc.sync.dma_start(palT[:], aps["palT"])
    galT = consts.tile([128, 14], bft)
    nc.sync.dma_start(galT[:], aps["galT"])

    for g in range(n_groups):
        lt = limb_pool.tile([KROWS, GPX], bft)
        nc.sync.dma_start(lt[:], aps["limbs"][:, g * GPX:(g + 1) * GPX])

        ps = psum_pool.tile([128, TPG * NCOL], f32)
        for t in range(TPG):
            nc.tensor.matmul(
                ps[:, t * NCOL:(t + 1) * NCOL],
                lt[:, t * PXT:(t + 1) * PXT],
                palT[:],
                start=True,
                stop=True,
            )

        ps3 = ps[:].rearrange("p (t k) -> p t k", k=NCOL)
        mx = small_pool.tile([128, TPG], f32)
        nc.vector.tensor_reduce(
            mx[:], ps3, axis=mybir.AxisListType.X, op=mybir.AluOpType.max
        )

        sadj = sadj_pool.tile([128, TPG * NCOL], f32)
        nc.vector.tensor_tensor(
            sadj[:].rearrange("p (t k) -> p t k", k=NCOL),
            ps3,
            mx[:, :, None].to_broadcast([128, TPG, NCOL]),
            mybir.AluOpType.subtract,
        )

        w = w_pool.tile([128, TPG * NCOL], bft)
        nc.scalar.activation(
            w[:], sadj[:], mybir.ActivationFunctionType.Exp, scale=BETA
        )

        # transpose w in (128,128) chunks: chunk j stacks px-tile pair
        # (2j, 2j+1) as [colors of 2j | colors of 2j+1] x 128 px
        wt = wt_pool.tile([128, TPG * NCOL], bft)
        for j in range(TPG // 2):
            nc.sync.dma_start_transpose(
                wt[:, j * 128:(j + 1) * 128], w[:, j * 128:(j + 1) * 128]
            )

        gp = gps_pool.tile([14, TPG // 2 * 128], f32)
        for j in range(TPG // 2):
            nc.tensor.matmul(
                gp[:, j * 128:(j + 1) * 128],
                galT[:],
                wt[:, j * 128:(j + 1) * 128],
                start=True,
                stop=True,
            )
        gs = small_pool.tile([14, TPG // 2 * 128], f32)
        nc.scalar.copy(gs[:], gp[:])
        nc.sync.dma_start(aps["feats"][g], gs[:])


def _build_nc(n_tiles):
    import concourse.bass as bass
    import concourse.mybir as mybir
    import concourse.tile as tile
    from concourse import bacc
    from contextlib import ExitStack

    hw = n_tiles * PXT
    nc = bacc.Bacc("TRN2", num_devices=8)
    aps = {
        "limbs": nc.dram_tensor(
            "limbs", (KROWS, hw), mybir.dt.bfloat16, kind="ExternalInput"
        ).ap(),
        "palT": nc.dram_tensor(
            "palT", (KROWS, NCOL), mybir.dt.bfloat16, kind="ExternalInput"
        ).ap(),
        "galT": nc.dram_tensor(
            "galT", (128, 14), mybir.dt.bfloat16, kind="ExternalInput"
        ).ap(),
        "feats": nc.dram_tensor(
            "feats", (n_tiles // TPG, 14, 512), mybir.dt.float32,
            kind="ExternalOutput",
        ).ap(),
    }
    with tile.TileContext(nc) as tc:
        with ExitStack() as ctx:
            _build_body(nc, tc, ctx, aps, n_tiles)
    nc.compile()
    return nc


def _get_nc():
    if "nc" not in _CACHE:
        _CACHE["nc"] = _build_nc(NTILES)
    return _CACHE["nc"]


def _host_inputs(x, palette):
    """x: (B,3,H,W) f32, palette: (64,3) f32 -> list of 8 per-core in-maps."""
    palT = _host_palette_rows(palette)
    galT = _host_palette_gather(palette)
    in_maps = []
    for b in range(B):
        px = np.asarray(x[b], np.float32).reshape(3, HW)
        in_maps.append(
            {"limbs": _host_pixel_limbs(px), "palT": palT, "galT": galT}
        )
    return in_maps


def _finish_core(f):
    """f: (n_groups, 14, 512) f32 -> (3, hw) f32 for one core.

    MM chunk j covers px-tile 8g+2j (rows 0-6) and 8g+2j+1 (rows 7-13);
    within a chunk, column m is pixel m of both tiles.
    """
    n_groups = f.shape[0]
    ff = f.reshape(n_groups, 2, 7, 4, 128)       # (g, s, feat, j, m)
    full = ff.transpose(0, 3, 1, 2, 4)           # (g, j, s, feat, m)
    full = full.reshape(n_groups * TPG, 7, 128)  # tile = 8g + 2j + s
    full = full.transpose(0, 2, 1).reshape(-1, 7)  # (hw, feat)
    cnt = full[:, 6]
    rgb = np.stack(
        [
            (full[:, 0] + full[:, 1]) / cnt,
            (full[:, 2] + full[:, 3]) / cnt,
            (full[:, 4] + full[:, 5]) / cnt,
        ],
        axis=0,
    )
    return rgb


def _host_finish(feats_list, palette):
    out = np.empty((B, 3, H, W), np.float32)
    for b, f in enumerate(feats_list):
        out[b] = _finish_core(np.asarray(f)).reshape(3, H, W)
    return out


def kernel(x, palette):
    from concourse.bass_utils import run_bass_kernel_spmd

    nc = _get_nc()
    in_maps = _host_inputs(x, palette)
    res = run_bass_kernel_spmd(nc, in_maps, core_ids=list(range(8)))
    feats = [res.results[i]["feats"] for i in range(B)]
    _CACHE["last_results"] = res
    return _host_finish(feats, palette).astype(np.float32)



# revision 10
# speedup vs baseline: 4.6907x; 4.6907x over previous
"""nn_ColorReducer — Trainium2 Bass kernel (8-core data-parallel).

Algorithm (exact nearest-palette-color via argmax-index extraction):
  score_k(p) = 2 p.c_k - ||c_k||^2  (maximizing k <=> nearest color)
  Scores on TensorE with 128-pixel tiles stationary (21 bf16 limb rows for
  ~2^-24 accuracy) and the palette moving, landing in PSUM as
  (128 px, 64 colors) — reducible along the free dim. Index extraction:
    mx  = reduce_max(scores)            [DVE]
    eq  = (scores >= mx) in bf16        [GpSimd]
    ni  = eq * (64 - k)                 [DVE, materialized reverse-iota]
    win = reduce_max(ni)                [DVE]  -> k* = 64 - win
  (reverse iota makes ties pick the smallest k, matching jnp.argmin).
  Only the 2-byte win indices are shipped out; the host gathers
  palette[k*] and reshapes.

Sharding: batch dim (8 images) across the 8 NeuronCores; palette replicated.
"""

import numpy as np
import ml_dtypes

bf16 = ml_dtypes.bfloat16

B, C, H, W = 8, 3, 512, 512
HW = H * W              # 262144 pixels per core
NCOL = 64               # palette entries
KROWS = 21              # limb product rows: 6 per channel + 3 bias rows
PXT = 128               # pixels per matmul tile (stationary columns)
TPG = 16                # matmul tiles per group (2 PSUM banks)
GPX = PXT * TPG         # 2048 pixels per group
NTILES = HW // PXT      # 2048
GROUPS = NTILES // TPG  # 128
GPC = 16                # groups per out-DMA chunk
CHUNKS = GROUPS // GPC  # 8

_CACHE = {}


def _split3(x):
    """f32 -> three bf16 limbs (h, m, l) with h+m+l ~ x to ~2^-24 rel."""
    x = np.asarray(x, np.float32)
    h = x.astype(bf16)
    r = x - h.astype(np.float32)
    m = r.astype(bf16)
    r2 = r - m.astype(np.float32)
    l = r2.astype(bf16)
    return h, m, l


def _host_pixel_limbs(px):
    """px: (3, HW) f32 -> (GROUPS, KROWS, GPX) bf16 stationary slabs.

    Per channel the 6 product rows pair pixel limbs with palette limbs:
      (ph,wh) (ph,wm) (pm,wh) (pm,wm) (ph,wl) (pl,wh)
    so the pixel side is [ph, ph, pm, pm, ph, pl]; rows 18-20 are ones
    (paired with the 3 bias limbs).
    """
    rows = np.empty((KROWS, px.shape[1]), dtype=bf16)
    for c in range(3):
        h, m, l = _split3(px[c])
        rows[6 * c + 0] = h
        rows[6 * c + 1] = h
        rows[6 * c + 2] = m
        rows[6 * c + 3] = m
        rows[6 * c + 4] = h
        rows[6 * c + 5] = l
    rows[18:21] = np.ones((3, px.shape[1]), dtype=bf16)
    return np.ascontiguousarray(
        rows.reshape(KROWS, GROUPS, GPX).transpose(1, 0, 2)
    )


def _host_palette_rows(palette):
    """palette: (64, 3) f32 -> (KROWS, 64) bf16 moving-operand rows."""
    pal = np.asarray(palette, np.float64)
    rows = np.empty((KROWS, NCOL), dtype=bf16)
    for c in range(3):
        wh, wm, wl = _split3((2.0 * pal[:, c]).astype(np.float32))
        rows[6 * c + 0] = wh
        rows[6 * c + 1] = wm
        rows[6 * c + 2] = wh
        rows[6 * c + 3] = wm
        rows[6 * c + 4] = wl
        rows[6 * c + 5] = wh
    b64 = -(pal ** 2).sum(axis=1)
    bh = b64.astype(np.float32).astype(bf16)
    r = b64 - bh.astype(np.float64)
    bm = r.astype(np.float32).astype(bf16)
    r2 = r - bm.astype(np.float64)
    bl = r2.astype(np.float32).astype(bf16)
    rows[18], rows[19], rows[20] = bh, bm, bl
    return rows


def _host_negiota():
    """(128, TPG*NCOL) bf16: value 64-k at color slot k of every tile."""
    row = (NCOL - np.arange(NCOL, dtype=np.float32)).astype(bf16)
    return np.ascontiguousarray(
        np.broadcast_to(np.tile(row, TPG), (PXT, TPG * NCOL))
    )


def _build_body(nc, tc, ctx, aps, n_tiles):
    """Emit the per-core program. aps: dict name->AP for dram tensors."""
    import concourse.bass as bass
    import concourse.mybir as mybir

    n_groups = n_tiles // TPG
    f32 = mybir.dt.float32
    bft = mybir.dt.bfloat16

    consts = ctx.enter_context(tc.tile_pool(name="consts", bufs=1))
    limb_pool = ctx.enter_context(tc.tile_pool(name="limb", bufs=4))
    psum_pool = ctx.enter_context(tc.tile_pool(name="psum", bufs=3, space="PSUM"))
    cp_pool = ctx.enter_context(tc.tile_pool(name="cp", bufs=3))
    mx_pool = ctx.enter_context(tc.tile_pool(name="mx", bufs=3))
    eq_pool = ctx.enter_context(tc.tile_pool(name="eq", bufs=3))
    ni_pool = ctx.enter_context(tc.tile_pool(name="ni", bufs=3))
    stage_pool = ctx.enter_context(tc.tile_pool(name="stage", bufs=2))

    palT = consts.tile([KROWS, NCOL], bft)
    nc.sync.dma_start(palT[:], aps["palT"])
    negiota = consts.tile([PXT, TPG * NCOL], bft)
    nc.sync.dma_start(negiota[:], aps["negiota"])

    stage = None
    for g in range(n_groups):
        if g % GPC == 0:
            stage = stage_pool.tile([128, GPC * TPG], bft)
        lt = limb_pool.tile([KROWS, GPX], bft)
        nc.sync.dma_start(lt[:], aps["limbs"][g])

        ps = psum_pool.tile([128, TPG * NCOL], f32)
        for t in range(TPG):
            nc.tensor.matmul(
                ps[:, t * NCOL:(t + 1) * NCOL],
                lt[:, t * PXT:(t + 1) * PXT],
                palT[:],
                start=True,
                stop=True,
            )

        cp = cp_pool.tile([128, TPG * NCOL], f32)
        nc.scalar.copy(cp[:], ps[:])

        cp3 = cp[:].rearrange("p (t k) -> p t k", k=NCOL)
        mx = mx_pool.tile([128, TPG], f32)
        nc.vector.tensor_reduce(
            mx[:], cp3, axis=mybir.AxisListType.X, op=mybir.AluOpType.max
        )

        eq = eq_pool.tile([128, TPG * NCOL], bft)
        nc.vector.tensor_tensor(
            eq[:].rearrange("p (t k) -> p t k", k=NCOL),
            cp3,
            mx[:, :, None].to_broadcast([128, TPG, NCOL]),
            mybir.AluOpType.is_ge,
        )

        ni = ni_pool.tile([128, TPG * NCOL], bft)
        nc.gpsimd.tensor_tensor(
            ni[:],
            eq[:],
            negiota[:],
            mybir.AluOpType.mult,
        )

        j = g % GPC
        nc.vector.tensor_reduce(
            stage[:, j * TPG:(j + 1) * TPG],
            ni[:].rearrange("p (t k) -> p t k", k=NCOL),
            axis=mybir.AxisListType.X,
            op=mybir.AluOpType.max,
        )
        if j == GPC - 1:
            nc.sync.dma_start(aps["win"][g // GPC], stage[:])


def _build_nc(n_tiles):
    import concourse.bass as bass
    import concourse.mybir as mybir
    import concourse.tile as tile
    from concourse import bacc
    from contextlib import ExitStack

    n_groups = n_tiles // TPG
    nc = bacc.Bacc("TRN2", num_devices=8)
    aps = {
        "limbs": nc.dram_tensor(
            "limbs", (n_groups, KROWS, GPX), mybir.dt.bfloat16,
            kind="ExternalInput"
        ).ap(),
        "palT": nc.dram_tensor(
            "palT", (KROWS, NCOL), mybir.dt.bfloat16, kind="ExternalInput"
        ).ap(),
        "negiota": nc.dram_tensor(
            "negiota", (PXT, TPG * NCOL), mybir.dt.bfloat16,
            kind="ExternalInput"
        ).ap(),
        "win": nc.dram_tensor(
            "win", (n_groups // GPC, 128, GPC * TPG), mybir.dt.bfloat16,
            kind="ExternalOutput",
        ).ap(),
    }
    with tile.TileContext(nc) as tc:
        with ExitStack() as ctx:
            _build_body(nc, tc, ctx, aps, n_tiles)
    nc.compile()
    return nc


def _get_nc():
    if "nc" not in _CACHE:
        _CACHE["nc"] = _build_nc(NTILES)
    return _CACHE["nc"]


def _host_inputs(x, palette):
    """x: (B,3,H,W) f32, palette: (64,3) f32 -> list of 8 per-core in-maps."""
    palT = _host_palette_rows(palette)
    negiota = _host_negiota()
    in_maps = []
    for b in range(B):
        px = np.asarray(x[b], np.float32).reshape(3, HW)
        in_maps.append(
            {
                "limbs": _host_pixel_limbs(px),
                "palT": palT,
                "negiota": negiota,
            }
        )
    return in_maps


def _host_finish(win_list, palette):
    """win: per-core (GROUPS, 128, TPG) bf16 -> (B,3,H,W) f32 output."""
    pal = np.asarray(palette, np.float32)
    out = np.empty((B, 3, H, W), np.float32)
    for b, wv in enumerate(win_list):
        w = np.asarray(wv).astype(np.float32)
        idx = (NCOL - w).astype(np.int64)               # (chunk, p, j*TPG+t)
        np.clip(idx, 0, NCOL - 1, out=idx)
        idx = idx.reshape(CHUNKS, 128, GPC, TPG)        # (c, p, j, t)
        idx = idx.transpose(0, 2, 3, 1).reshape(HW)     # (c, j, t, p) -> px
        out[b] = pal[idx].T.reshape(3, H, W)
    return out


def kernel(x, palette):
    from concourse.bass_utils import run_bass_kernel_spmd

    nc = _get_nc()
    in_maps = _host_inputs(x, palette)
    res = run_bass_kernel_spmd(nc, in_maps, core_ids=list(range(8)))
    win = [res.results[i]["win"] for i in range(B)]
    _CACHE["last_results"] = res
    return _host_finish(win, palette).astype(np.float32)


# revision 13
# speedup vs baseline: 4.9199x; 1.0489x over previous
"""nn_ColorReducer — Trainium2 Bass kernel (8-core data-parallel).

Algorithm (exact nearest-palette-color via argmax-index extraction):
  score_k(p) = 2 p.c_k - ||c_k||^2  (maximizing k <=> nearest color)
  Scores on TensorE with 128-pixel tiles stationary (21 bf16 limb rows for
  ~2^-24 accuracy) and the palette moving, landing in PSUM as
  (128 px, 64 colors) — reducible along the free dim. Index extraction:
    mx  = reduce_max(scores)            [DVE]
    eq  = (scores >= mx) in bf16        [GpSimd]
    ni  = eq * (64 - k)                 [DVE, materialized reverse-iota]
    win = reduce_max(ni)                [DVE]  -> k* = 64 - win
  (reverse iota makes ties pick the smallest k, matching jnp.argmin).
  Only the 2-byte win indices are shipped out; the host gathers
  palette[k*] and reshapes.

Sharding: batch dim (8 images) across the 8 NeuronCores; palette replicated.
"""

import numpy as np
import ml_dtypes

bf16 = ml_dtypes.bfloat16

B, C, H, W = 8, 3, 512, 512
HW = H * W              # 262144 pixels per core
NCOL = 64               # palette entries
KROWS = 21              # limb product rows: 6 per channel + 3 bias rows
PXT = 128               # pixels per matmul tile (stationary columns)
TPG = 16                # matmul tiles per group (2 PSUM banks)
GPX = PXT * TPG         # 2048 pixels per group
NTILES = HW // PXT      # 2048
GROUPS = NTILES // TPG  # 128
GPC = 16                # groups per out-DMA chunk
CHUNKS = GROUPS // GPC  # 8

_CACHE = {}


def _split3(x):
    """f32 -> three bf16 limbs (h, m, l) with h+m+l ~ x to ~2^-24 rel."""
    x = np.asarray(x, np.float32)
    h = x.astype(bf16)
    r = x - h.astype(np.float32)
    m = r.astype(bf16)
    r2 = r - m.astype(np.float32)
    l = r2.astype(bf16)
    return h, m, l


def _host_pixel_limbs(px):
    """px: (3, HW) f32 -> (GROUPS, KROWS, GPX) bf16 stationary slabs.

    Per channel the 6 product rows pair pixel limbs with palette limbs:
      (ph,wh) (ph,wm) (pm,wh) (pm,wm) (ph,wl) (pl,wh)
    so the pixel side is [ph, ph, pm, pm, ph, pl]; rows 18-20 are ones
    (paired with the 3 bias limbs).
    """
    rows = np.empty((KROWS, px.shape[1]), dtype=bf16)
    for c in range(3):
        h, m, l = _split3(px[c])
        rows[6 * c + 0] = h
        rows[6 * c + 1] = h
        rows[6 * c + 2] = m
        rows[6 * c + 3] = m
        rows[6 * c + 4] = h
        rows[6 * c + 5] = l
    rows[18:21] = np.ones((3, px.shape[1]), dtype=bf16)
    return np.ascontiguousarray(
        rows.reshape(KROWS, GROUPS, GPX).transpose(1, 0, 2)
    )


def _host_palette_rows(palette):
    """palette: (64, 3) f32 -> (KROWS, 64) bf16 moving-operand rows."""
    pal = np.asarray(palette, np.float64)
    rows = np.empty((KROWS, NCOL), dtype=bf16)
    for c in range(3):
        wh, wm, wl = _split3((2.0 * pal[:, c]).astype(np.float32))
        rows[6 * c + 0] = wh
        rows[6 * c + 1] = wm
        rows[6 * c + 2] = wh
        rows[6 * c + 3] = wm
        rows[6 * c + 4] = wl
        rows[6 * c + 5] = wh
    b64 = -(pal ** 2).sum(axis=1)
    bh = b64.astype(np.float32).astype(bf16)
    r = b64 - bh.astype(np.float64)
    bm = r.astype(np.float32).astype(bf16)
    r2 = r - bm.astype(np.float64)
    bl = r2.astype(np.float32).astype(bf16)
    rows[18], rows[19], rows[20] = bh, bm, bl
    return rows


def _host_negiota():
    """(128, TPG*NCOL) bf16: value 64-k at color slot k of every tile."""
    row = (NCOL - np.arange(NCOL, dtype=np.float32)).astype(bf16)
    return np.ascontiguousarray(
        np.broadcast_to(np.tile(row, TPG), (PXT, TPG * NCOL))
    )


def _build_body(nc, tc, ctx, aps, n_tiles):
    """Emit the per-core program. aps: dict name->AP for dram tensors."""
    import concourse.bass as bass
    import concourse.mybir as mybir

    n_groups = n_tiles // TPG
    f32 = mybir.dt.float32
    bft = mybir.dt.bfloat16

    consts = ctx.enter_context(tc.tile_pool(name="consts", bufs=1))
    limb_pool = ctx.enter_context(tc.tile_pool(name="limb", bufs=4))
    psum_pool = ctx.enter_context(tc.tile_pool(name="psum", bufs=3, space="PSUM"))
    cp_pool = ctx.enter_context(tc.tile_pool(name="cp", bufs=3))
    mx_pool = ctx.enter_context(tc.tile_pool(name="mx", bufs=3))
    eq_pool = ctx.enter_context(tc.tile_pool(name="eq", bufs=3))
    ni_pool = ctx.enter_context(tc.tile_pool(name="ni", bufs=3))
    stage_pool = ctx.enter_context(tc.tile_pool(name="stage", bufs=2))

    palT = consts.tile([KROWS, NCOL], bft)
    nc.sync.dma_start(palT[:], aps["palT"])
    negiota = consts.tile([PXT, TPG * NCOL], bft)
    nc.sync.dma_start(negiota[:], aps["negiota"])

    stages = {}
    ni_tiles = {}

    def emit_i2(h):
        """Deferred final reduce for group h: ni -> win slice, + chunk DMA."""
        st = stages[h // GPC]
        j = h % GPC
        nc.vector.tensor_reduce(
            st[:, j * TPG:(j + 1) * TPG],
            ni_tiles.pop(h)[:].rearrange("p (t k) -> p t k", k=NCOL),
            axis=mybir.AxisListType.X,
            op=mybir.AluOpType.max,
        )
        if j == GPC - 1:
            nc.sync.dma_start(aps["win"][h // GPC], st[:])
            del stages[h // GPC]

    for g in range(n_groups):
        if g % GPC == 0:
            stages[g // GPC] = stage_pool.tile(
                [128, GPC * TPG], bft, name=f"stage{g // GPC}"
            )
        lt = limb_pool.tile([KROWS, GPX], bft)
        nc.sync.dma_start(lt[:], aps["limbs"][g])

        ps = psum_pool.tile([128, TPG * NCOL], f32)
        for t in range(TPG):
            nc.tensor.matmul(
                ps[:, t * NCOL:(t + 1) * NCOL],
                lt[:, t * PXT:(t + 1) * PXT],
                palT[:],
                start=True,
                stop=True,
            )

        cp = cp_pool.tile([128, TPG * NCOL], f32)
        nc.scalar.copy(cp[:], ps[:])

        cp3 = cp[:].rearrange("p (t k) -> p t k", k=NCOL)
        mx = mx_pool.tile([128, TPG], f32)
        nc.vector.tensor_reduce(
            mx[:], cp3, axis=mybir.AxisListType.X, op=mybir.AluOpType.max
        )

        eq = eq_pool.tile([128, TPG * NCOL], bft)
        nc.vector.tensor_tensor(
            eq[:].rearrange("p (t k) -> p t k", k=NCOL),
            cp3,
            mx[:, :, None].to_broadcast([128, TPG, NCOL]),
            mybir.AluOpType.is_ge,
        )

        ni = ni_pool.tile([128, TPG * NCOL], bft)
        nc.gpsimd.tensor_tensor(
            ni[:],
            eq[:],
            negiota[:],
            mybir.AluOpType.mult,
        )
        ni_tiles[g] = ni

        # software-pipeline: the final reduce for group g-1 runs now, so
        # the in-order DVE queue never stalls waiting on GpSimd's multiply
        if g > 0:
            emit_i2(g - 1)
    emit_i2(n_groups - 1)


def _build_nc(n_tiles):
    import concourse.bass as bass
    import concourse.mybir as mybir
    import concourse.tile as tile
    from concourse import bacc
    from contextlib import ExitStack

    n_groups = n_tiles // TPG
    nc = bacc.Bacc("TRN2", num_devices=8)
    aps = {
        "limbs": nc.dram_tensor(
            "limbs", (n_groups, KROWS, GPX), mybir.dt.bfloat16,
            kind="ExternalInput"
        ).ap(),
        "palT": nc.dram_tensor(
            "palT", (KROWS, NCOL), mybir.dt.bfloat16, kind="ExternalInput"
        ).ap(),
        "negiota": nc.dram_tensor(
            "negiota", (PXT, TPG * NCOL), mybir.dt.bfloat16,
            kind="ExternalInput"
        ).ap(),
        "win": nc.dram_tensor(
            "win", (n_groups // GPC, 128, GPC * TPG), mybir.dt.bfloat16,
            kind="ExternalOutput",
        ).ap(),
    }
    with tile.TileContext(nc) as tc:
        with ExitStack() as ctx:
            _build_body(nc, tc, ctx, aps, n_tiles)
    nc.compile()
    return nc


def _get_nc():
    if "nc" not in _CACHE:
        _CACHE["nc"] = _build_nc(NTILES)
    return _CACHE["nc"]


def _host_inputs(x, palette):
    """x: (B,3,H,W) f32, palette: (64,3) f32 -> list of 8 per-core in-maps."""
    palT = _host_palette_rows(palette)
    negiota = _host_negiota()
    in_maps = []
    for b in range(B):
        px = np.asarray(x[b], np.float32).reshape(3, HW)
        in_maps.append(
            {
                "limbs": _host_pixel_limbs(px),
                "palT": palT,
                "negiota": negiota,
            }
        )
    return in_maps


def _host_finish(win_list, palette):
    """win: per-core (GROUPS, 128, TPG) bf16 -> (B,3,H,W) f32 output."""
    pal = np.asarray(palette, np.float32)
    out = np.empty((B, 3, H, W), np.float32)
    for b, wv in enumerate(win_list):
        w = np.asarray(wv).astype(np.float32)
        idx = (NCOL - w).astype(np.int64)               # (chunk, p, j*TPG+t)
        np.clip(idx, 0, NCOL - 1, out=idx)
        idx = idx.reshape(CHUNKS, 128, GPC, TPG)        # (c, p, j, t)
        idx = idx.transpose(0, 2, 3, 1).reshape(HW)     # (c, j, t, p) -> px
        out[b] = pal[idx].T.reshape(3, H, W)
    return out


def kernel(x, palette):
    from concourse.bass_utils import run_bass_kernel_spmd

    nc = _get_nc()
    in_maps = _host_inputs(x, palette)
    res = run_bass_kernel_spmd(nc, in_maps, core_ids=list(range(8)))
    win = [res.results[i]["win"] for i in range(B)]
    _CACHE["last_results"] = res
    return _host_finish(win, palette).astype(np.float32)


# revision 15
# speedup vs baseline: 4.9259x; 1.0012x over previous
"""nn_ColorReducer — Trainium2 Bass kernel (8-core data-parallel).

Algorithm (exact nearest-palette-color via argmax-index extraction):
  score_k(p) = 2 p.c_k - ||c_k||^2  (maximizing k <=> nearest color)
  Scores on TensorE with 128-pixel tiles stationary (21 bf16 limb rows for
  ~2^-24 accuracy) and the palette moving, landing in PSUM as
  (128 px, 64 colors) — reducible along the free dim. Index extraction:
    mx  = reduce_max(scores)            [DVE]
    eq  = (scores >= mx) in bf16        [GpSimd]
    ni  = eq * (64 - k)                 [DVE, materialized reverse-iota]
    win = reduce_max(ni)                [DVE]  -> k* = 64 - win
  (reverse iota makes ties pick the smallest k, matching jnp.argmin).
  Only the 2-byte win indices are shipped out; the host gathers
  palette[k*] and reshapes.

Sharding: batch dim (8 images) across the 8 NeuronCores; palette replicated.
"""

import numpy as np
import ml_dtypes

bf16 = ml_dtypes.bfloat16

B, C, H, W = 8, 3, 512, 512
HW = H * W              # 262144 pixels per core
NCOL = 64               # palette entries
KROWS = 21              # limb product rows: 6 per channel + 3 bias rows
PXT = 128               # pixels per matmul tile (stationary columns)
TPG = 16                # matmul tiles per group (2 PSUM banks)
GPX = PXT * TPG         # 2048 pixels per group
NTILES = HW // PXT      # 2048
GROUPS = NTILES // TPG  # 128
GPC = 16                # groups per out-DMA chunk
CHUNKS = GROUPS // GPC  # 8

_CACHE = {}


def _split3(x):
    """f32 -> three bf16 limbs (h, m, l) with h+m+l ~ x to ~2^-24 rel."""
    x = np.asarray(x, np.float32)
    h = x.astype(bf16)
    r = x - h.astype(np.float32)
    m = r.astype(bf16)
    r2 = r - m.astype(np.float32)
    l = r2.astype(bf16)
    return h, m, l


def _host_pixel_limbs(px):
    """px: (3, HW) f32 -> (GROUPS, KROWS, GPX) bf16 stationary slabs.

    Per channel the 6 product rows pair pixel limbs with palette limbs:
      (ph,wh) (ph,wm) (pm,wh) (pm,wm) (ph,wl) (pl,wh)
    so the pixel side is [ph, ph, pm, pm, ph, pl]; rows 18-20 are ones
    (paired with the 3 bias limbs).
    """
    rows = np.empty((KROWS, px.shape[1]), dtype=bf16)
    for c in range(3):
        h, m, l = _split3(px[c])
        rows[6 * c + 0] = h
        rows[6 * c + 1] = h
        rows[6 * c + 2] = m
        rows[6 * c + 3] = m
        rows[6 * c + 4] = h
        rows[6 * c + 5] = l
    rows[18:21] = np.ones((3, px.shape[1]), dtype=bf16)
    return np.ascontiguousarray(
        rows.reshape(KROWS, GROUPS, GPX).transpose(1, 0, 2)
    )


def _host_palette_rows(palette):
    """palette: (64, 3) f32 -> (KROWS, 64) bf16 moving-operand rows."""
    pal = np.asarray(palette, np.float64)
    rows = np.empty((KROWS, NCOL), dtype=bf16)
    for c in range(3):
        wh, wm, wl = _split3((2.0 * pal[:, c]).astype(np.float32))
        rows[6 * c + 0] = wh
        rows[6 * c + 1] = wm
        rows[6 * c + 2] = wh
        rows[6 * c + 3] = wm
        rows[6 * c + 4] = wl
        rows[6 * c + 5] = wh
    b64 = -(pal ** 2).sum(axis=1)
    bh = b64.astype(np.float32).astype(bf16)
    r = b64 - bh.astype(np.float64)
    bm = r.astype(np.float32).astype(bf16)
    r2 = r - bm.astype(np.float64)
    bl = r2.astype(np.float32).astype(bf16)
    rows[18], rows[19], rows[20] = bh, bm, bl
    return rows


def _host_negiota():
    """(128, TPG*NCOL) bf16: value 64-k at color slot k of every tile."""
    row = (NCOL - np.arange(NCOL, dtype=np.float32)).astype(bf16)
    return np.ascontiguousarray(
        np.broadcast_to(np.tile(row, TPG), (PXT, TPG * NCOL))
    )


def _build_body(nc, tc, ctx, aps, n_tiles):
    """Emit the per-core program. aps: dict name->AP for dram tensors."""
    import concourse.bass as bass
    import concourse.mybir as mybir

    n_groups = n_tiles // TPG
    f32 = mybir.dt.float32
    bft = mybir.dt.bfloat16

    consts = ctx.enter_context(tc.tile_pool(name="consts", bufs=1))
    limb_pool = ctx.enter_context(tc.tile_pool(name="limb", bufs=4))
    psum_pool = ctx.enter_context(tc.tile_pool(name="psum", bufs=3, space="PSUM"))
    cp_pool = ctx.enter_context(tc.tile_pool(name="cp", bufs=4))
    mx_pool = ctx.enter_context(tc.tile_pool(name="mx", bufs=3))
    eq_pool = ctx.enter_context(tc.tile_pool(name="eq", bufs=3))
    ni_pool = ctx.enter_context(tc.tile_pool(name="ni", bufs=3))
    stage_pool = ctx.enter_context(tc.tile_pool(name="stage", bufs=2))

    palT = consts.tile([KROWS, NCOL], bft)
    nc.sync.dma_start(palT[:], aps["palT"])
    negiota = consts.tile([PXT, TPG * NCOL], bft)
    nc.sync.dma_start(negiota[:], aps["negiota"])

    stages = {}
    cp_tiles = {}
    ni_tiles = {}

    def emit_consume(h):
        """R/E (DVE) + payload mult (GpSimd) for group h, one group after
        its producers — the ~1.1us Scalar->Vector semaphore/drain latency
        then hides behind the previous group's DVE work."""
        cp3 = cp_tiles.pop(h)[:].rearrange("p (t k) -> p t k", k=NCOL)
        mx = mx_pool.tile([128, TPG], f32, name=f"mx{h % 4}")
        nc.vector.tensor_reduce(
            mx[:], cp3, axis=mybir.AxisListType.X, op=mybir.AluOpType.max
        )
        eq = eq_pool.tile([128, TPG * NCOL], bft, name=f"eq{h % 4}")
        nc.vector.tensor_tensor(
            eq[:].rearrange("p (t k) -> p t k", k=NCOL),
            cp3,
            mx[:, :, None].to_broadcast([128, TPG, NCOL]),
            mybir.AluOpType.is_ge,
        )
        ni = ni_pool.tile([128, TPG * NCOL], bft, name=f"ni{h % 4}")
        nc.gpsimd.tensor_tensor(
            ni[:], eq[:], negiota[:], mybir.AluOpType.mult
        )
        ni_tiles[h] = ni

    def emit_i2(h):
        """Final reduce for group h: ni -> win slice, + chunk DMA."""
        st = stages[h // GPC]
        j = h % GPC
        nc.vector.tensor_reduce(
            st[:, j * TPG:(j + 1) * TPG],
            ni_tiles.pop(h)[:].rearrange("p (t k) -> p t k", k=NCOL),
            axis=mybir.AxisListType.X,
            op=mybir.AluOpType.max,
        )
        if j == GPC - 1:
            nc.sync.dma_start(aps["win"][h // GPC], st[:])
            del stages[h // GPC]

    for g in range(n_groups):
        if g % GPC == 0:
            stages[g // GPC] = stage_pool.tile(
                [128, GPC * TPG], bft, name=f"stage{g // GPC}"
            )
        lt = limb_pool.tile([KROWS, GPX], bft)
        nc.sync.dma_start(lt[:], aps["limbs"][g])

        ps = psum_pool.tile([128, TPG * NCOL], f32)
        for t in range(TPG):
            nc.tensor.matmul(
                ps[:, t * NCOL:(t + 1) * NCOL],
                lt[:, t * PXT:(t + 1) * PXT],
                palT[:],
                start=True,
                stop=True,
            )

        cp = cp_pool.tile([128, TPG * NCOL], f32, name=f"cp{g % 4}")
        nc.scalar.copy(cp[:], ps[:])
        cp_tiles[g] = cp

        if g > 0:
            emit_consume(g - 1)
        if g > 1:
            emit_i2(g - 2)
    emit_consume(n_groups - 1)
    emit_i2(n_groups - 2)
    emit_i2(n_groups - 1)


def _build_nc(n_tiles):
    import concourse.bass as bass
    import concourse.mybir as mybir
    import concourse.tile as tile
    from concourse import bacc
    from contextlib import ExitStack

    n_groups = n_tiles // TPG
    nc = bacc.Bacc("TRN2", num_devices=8)
    aps = {
        "limbs": nc.dram_tensor(
            "limbs", (n_groups, KROWS, GPX), mybir.dt.bfloat16,
            kind="ExternalInput"
        ).ap(),
        "palT": nc.dram_tensor(
            "palT", (KROWS, NCOL), mybir.dt.bfloat16, kind="ExternalInput"
        ).ap(),
        "negiota": nc.dram_tensor(
            "negiota", (PXT, TPG * NCOL), mybir.dt.bfloat16,
            kind="ExternalInput"
        ).ap(),
        "win": nc.dram_tensor(
            "win", (n_groups // GPC, 128, GPC * TPG), mybir.dt.bfloat16,
            kind="ExternalOutput",
        ).ap(),
    }
    with tile.TileContext(nc) as tc:
        with ExitStack() as ctx:
            _build_body(nc, tc, ctx, aps, n_tiles)
    nc.compile()
    return nc


def _get_nc():
    if "nc" not in _CACHE:
        _CACHE["nc"] = _build_nc(NTILES)
    return _CACHE["nc"]


def _host_inputs(x, palette):
    """x: (B,3,H,W) f32, palette: (64,3) f32 -> list of 8 per-core in-maps."""
    palT = _host_palette_rows(palette)
    negiota = _host_negiota()
    in_maps = []
    for b in range(B):
        px = np.asarray(x[b], np.float32).reshape(3, HW)
        in_maps.append(
            {
                "limbs": _host_pixel_limbs(px),
                "palT": palT,
                "negiota": negiota,
            }
        )
    return in_maps


def _host_finish(win_list, palette):
    """win: per-core (GROUPS, 128, TPG) bf16 -> (B,3,H,W) f32 output."""
    pal = np.asarray(palette, np.float32)
    out = np.empty((B, 3, H, W), np.float32)
    for b, wv in enumerate(win_list):
        w = np.asarray(wv).astype(np.float32)
        idx = (NCOL - w).astype(np.int64)               # (chunk, p, j*TPG+t)
        np.clip(idx, 0, NCOL - 1, out=idx)
        idx = idx.reshape(CHUNKS, 128, GPC, TPG)        # (c, p, j, t)
        idx = idx.transpose(0, 2, 3, 1).reshape(HW)     # (c, j, t, p) -> px
        out[b] = pal[idx].T.reshape(3, H, W)
    return out


def kernel(x, palette):
    from concourse.bass_utils import run_bass_kernel_spmd

    nc = _get_nc()
    in_maps = _host_inputs(x, palette)
    res = run_bass_kernel_spmd(nc, in_maps, core_ids=list(range(8)))
    win = [res.results[i]["win"] for i in range(B)]
    _CACHE["last_results"] = res
    return _host_finish(win, palette).astype(np.float32)


# revision 17
# speedup vs baseline: 4.9397x; 1.0028x over previous
"""nn_ColorReducer — Trainium2 Bass kernel (8-core data-parallel).

Algorithm (exact nearest-palette-color via argmax-index extraction):
  score_k(p) = 2 p.c_k - ||c_k||^2  (maximizing k <=> nearest color)
  Scores on TensorE with 128-pixel tiles stationary (21 bf16 limb rows for
  ~2^-24 accuracy) and the palette moving, landing in PSUM as
  (128 px, 64 colors) — reducible along the free dim. Index extraction:
    mx  = reduce_max(scores)            [DVE]
    eq  = (scores >= mx) in bf16        [GpSimd]
    ni  = eq * (64 - k)                 [DVE, materialized reverse-iota]
    win = reduce_max(ni)                [DVE]  -> k* = 64 - win
  (reverse iota makes ties pick the smallest k, matching jnp.argmin).
  Only the 2-byte win indices are shipped out; the host gathers
  palette[k*] and reshapes.

Sharding: batch dim (8 images) across the 8 NeuronCores; palette replicated.
"""

import numpy as np
import ml_dtypes

bf16 = ml_dtypes.bfloat16

B, C, H, W = 8, 3, 512, 512
HW = H * W              # 262144 pixels per core
NCOL = 64               # palette entries
KROWS = 21              # limb product rows: 6 per channel + 3 bias rows
PXT = 128               # pixels per matmul tile (stationary columns)
TPG = 16                # matmul tiles per group (2 PSUM banks)
GPX = PXT * TPG         # 2048 pixels per group
NTILES = HW // PXT      # 2048
GROUPS = NTILES // TPG  # 128
GPC = 16                # groups per out-DMA chunk
CHUNKS = GROUPS // GPC  # 8

_CACHE = {}


def _split3(x):
    """f32 -> three bf16 limbs (h, m, l) with h+m+l ~ x to ~2^-24 rel."""
    x = np.asarray(x, np.float32)
    h = x.astype(bf16)
    r = x - h.astype(np.float32)
    m = r.astype(bf16)
    r2 = r - m.astype(np.float32)
    l = r2.astype(bf16)
    return h, m, l


def _host_pixel_limbs(px):
    """px: (3, HW) f32 -> (GROUPS, KROWS, GPX) bf16 stationary slabs.

    Per channel the 6 product rows pair pixel limbs with palette limbs:
      (ph,wh) (ph,wm) (pm,wh) (pm,wm) (ph,wl) (pl,wh)
    so the pixel side is [ph, ph, pm, pm, ph, pl]; rows 18-20 are ones
    (paired with the 3 bias limbs).
    """
    rows = np.empty((KROWS, px.shape[1]), dtype=bf16)
    for c in range(3):
        h, m, l = _split3(px[c])
        rows[6 * c + 0] = h
        rows[6 * c + 1] = h
        rows[6 * c + 2] = m
        rows[6 * c + 3] = m
        rows[6 * c + 4] = h
        rows[6 * c + 5] = l
    rows[18:21] = np.ones((3, px.shape[1]), dtype=bf16)
    return np.ascontiguousarray(
        rows.reshape(KROWS, GROUPS, GPX).transpose(1, 0, 2)
    )


def _host_palette_rows(palette):
    """palette: (64, 3) f32 -> (KROWS, 64) bf16 moving-operand rows."""
    pal = np.asarray(palette, np.float64)
    rows = np.empty((KROWS, NCOL), dtype=bf16)
    for c in range(3):
        wh, wm, wl = _split3((2.0 * pal[:, c]).astype(np.float32))
        rows[6 * c + 0] = wh
        rows[6 * c + 1] = wm
        rows[6 * c + 2] = wh
        rows[6 * c + 3] = wm
        rows[6 * c + 4] = wl
        rows[6 * c + 5] = wh
    b64 = -(pal ** 2).sum(axis=1)
    bh = b64.astype(np.float32).astype(bf16)
    r = b64 - bh.astype(np.float64)
    bm = r.astype(np.float32).astype(bf16)
    r2 = r - bm.astype(np.float64)
    bl = r2.astype(np.float32).astype(bf16)
    rows[18], rows[19], rows[20] = bh, bm, bl
    return rows


def _host_negiota():
    """(128, TPG*NCOL) bf16: value 64-k at color slot k of every tile."""
    row = (NCOL - np.arange(NCOL, dtype=np.float32)).astype(bf16)
    return np.ascontiguousarray(
        np.broadcast_to(np.tile(row, TPG), (PXT, TPG * NCOL))
    )


def _build_body(nc, tc, ctx, aps, n_tiles):
    """Emit the per-core program. aps: dict name->AP for dram tensors."""
    import concourse.bass as bass
    import concourse.mybir as mybir

    n_groups = n_tiles // TPG
    f32 = mybir.dt.float32
    bft = mybir.dt.bfloat16

    consts = ctx.enter_context(tc.tile_pool(name="consts", bufs=1))
    limb_pool = ctx.enter_context(tc.tile_pool(name="limb", bufs=4))
    psum_pool = ctx.enter_context(tc.tile_pool(name="psum", bufs=4, space="PSUM"))
    cp_pool = ctx.enter_context(tc.tile_pool(name="cp", bufs=5))
    mx_pool = ctx.enter_context(tc.tile_pool(name="mx", bufs=4))
    eq_pool = ctx.enter_context(tc.tile_pool(name="eq", bufs=4))
    ni_pool = ctx.enter_context(tc.tile_pool(name="ni", bufs=4))
    stage_pool = ctx.enter_context(tc.tile_pool(name="stage", bufs=2))

    palT = consts.tile([KROWS, NCOL], bft)
    nc.sync.dma_start(palT[:], aps["palT"])
    negiota = consts.tile([PXT, TPG * NCOL], bft)
    nc.sync.dma_start(negiota[:], aps["negiota"])

    stages = {}
    cp_tiles = {}
    ni_tiles = {}

    def emit_consume(h):
        """R/E (DVE) + payload mult (GpSimd) for group h, one group after
        its producers — the ~1.1us Scalar->Vector semaphore/drain latency
        then hides behind the previous group's DVE work."""
        cp3 = cp_tiles.pop(h)[:].rearrange("p (t k) -> p t k", k=NCOL)
        mx = mx_pool.tile([128, TPG], f32, name="mx")
        nc.vector.tensor_reduce(
            mx[:], cp3, axis=mybir.AxisListType.X, op=mybir.AluOpType.max
        )
        eq = eq_pool.tile([128, TPG * NCOL], bft, name="eq")
        nc.vector.tensor_tensor(
            eq[:].rearrange("p (t k) -> p t k", k=NCOL),
            cp3,
            mx[:, :, None].to_broadcast([128, TPG, NCOL]),
            mybir.AluOpType.is_ge,
        )
        ni = ni_pool.tile([128, TPG * NCOL], bft, name="ni")
        nc.gpsimd.tensor_tensor(
            ni[:], eq[:], negiota[:], mybir.AluOpType.mult
        )
        ni_tiles[h] = ni

    def emit_i2(h):
        """Final reduce for group h: ni -> win slice, + chunk DMA."""
        st = stages[h // GPC]
        j = h % GPC
        nc.vector.tensor_reduce(
            st[:, j * TPG:(j + 1) * TPG],
            ni_tiles.pop(h)[:].rearrange("p (t k) -> p t k", k=NCOL),
            axis=mybir.AxisListType.X,
            op=mybir.AluOpType.max,
        )
        if j == GPC - 1:
            nc.sync.dma_start(aps["win"][h // GPC], st[:])
            del stages[h // GPC]

    for g in range(n_groups):
        if g % GPC == 0:
            stages[g // GPC] = stage_pool.tile(
                [128, GPC * TPG], bft, name="stage"
            )
        lt = limb_pool.tile([KROWS, GPX], bft)
        nc.sync.dma_start(lt[:], aps["limbs"][g])

        ps = psum_pool.tile([128, TPG * NCOL], f32)
        for t in range(TPG):
            nc.tensor.matmul(
                ps[:, t * NCOL:(t + 1) * NCOL],
                lt[:, t * PXT:(t + 1) * PXT],
                palT[:],
                start=True,
                stop=True,
            )

        cp = cp_pool.tile([128, TPG * NCOL], f32, name="cp")
        nc.scalar.copy(cp[:], ps[:])
        cp_tiles[g] = cp

        if g > 1:
            emit_consume(g - 2)
        if g > 2:
            emit_i2(g - 3)
    emit_consume(n_groups - 2)
    emit_consume(n_groups - 1)
    emit_i2(n_groups - 3)
    emit_i2(n_groups - 2)
    emit_i2(n_groups - 1)


def _build_nc(n_tiles):
    import concourse.bass as bass
    import concourse.mybir as mybir
    import concourse.tile as tile
    from concourse import bacc
    from contextlib import ExitStack

    n_groups = n_tiles // TPG
    nc = bacc.Bacc("TRN2", num_devices=8)
    aps = {
        "limbs": nc.dram_tensor(
            "limbs", (n_groups, KROWS, GPX), mybir.dt.bfloat16,
            kind="ExternalInput"
        ).ap(),
        "palT": nc.dram_tensor(
            "palT", (KROWS, NCOL), mybir.dt.bfloat16, kind="ExternalInput"
        ).ap(),
        "negiota": nc.dram_tensor(
            "negiota", (PXT, TPG * NCOL), mybir.dt.bfloat16,
            kind="ExternalInput"
        ).ap(),
        "win": nc.dram_tensor(
            "win", (n_groups // GPC, 128, GPC * TPG), mybir.dt.bfloat16,
            kind="ExternalOutput",
        ).ap(),
    }
    with tile.TileContext(nc) as tc:
        with ExitStack() as ctx:
            _build_body(nc, tc, ctx, aps, n_tiles)
    nc.compile()
    return nc


def _get_nc():
    if "nc" not in _CACHE:
        _CACHE["nc"] = _build_nc(NTILES)
    return _CACHE["nc"]


def _host_inputs(x, palette):
    """x: (B,3,H,W) f32, palette: (64,3) f32 -> list of 8 per-core in-maps."""
    palT = _host_palette_rows(palette)
    negiota = _host_negiota()
    in_maps = []
    for b in range(B):
        px = np.asarray(x[b], np.float32).reshape(3, HW)
        in_maps.append(
            {
                "limbs": _host_pixel_limbs(px),
                "palT": palT,
                "negiota": negiota,
            }
        )
    return in_maps


def _host_finish(win_list, palette):
    """win: per-core (GROUPS, 128, TPG) bf16 -> (B,3,H,W) f32 output."""
    pal = np.asarray(palette, np.float32)
    out = np.empty((B, 3, H, W), np.float32)
    for b, wv in enumerate(win_list):
        w = np.asarray(wv).astype(np.float32)
        idx = (NCOL - w).astype(np.int64)               # (chunk, p, j*TPG+t)
        np.clip(idx, 0, NCOL - 1, out=idx)
        idx = idx.reshape(CHUNKS, 128, GPC, TPG)        # (c, p, j, t)
        idx = idx.transpose(0, 2, 3, 1).reshape(HW)     # (c, j, t, p) -> px
        out[b] = pal[idx].T.reshape(3, H, W)
    return out


def kernel(x, palette):
    from concourse.bass_utils import run_bass_kernel_spmd

    nc = _get_nc()
    in_maps = _host_inputs(x, palette)
    res = run_bass_kernel_spmd(nc, in_maps, core_ids=list(range(8)))
    win = [res.results[i]["win"] for i in range(B)]
    _CACHE["last_results"] = res
    return _host_finish(win, palette).astype(np.float32)


# revision 18
# speedup vs baseline: 6.6839x; 1.3531x over previous
"""nn_ColorReducer — Trainium2 Bass kernel (8-core data-parallel).

Algorithm (exact nearest-palette-color via argmax-index extraction):
  score_k(p) = 2 p.c_k - ||c_k||^2  (maximizing k <=> nearest color)
  Scores on TensorE with 128-pixel tiles stationary (21 bf16 limb rows for
  ~2^-24 accuracy) and the palette moving, landing in PSUM as
  (128 px, 64 colors) — reducible along the free dim. Index extraction:
    mx  = reduce_max(scores)            [DVE]
    eq  = (scores >= mx) in bf16        [GpSimd]
    ni  = eq * (64 - k)                 [DVE, materialized reverse-iota]
    win = reduce_max(ni)                [DVE]  -> k* = 64 - win
  (reverse iota makes ties pick the smallest k, matching jnp.argmin).
  Only the 2-byte win indices are shipped out; the host gathers
  palette[k*] and reshapes.

Sharding: batch dim (8 images) across the 8 NeuronCores; palette replicated.
"""

import numpy as np
import ml_dtypes

bf16 = ml_dtypes.bfloat16

B, C, H, W = 8, 3, 512, 512
HW = H * W              # 262144 pixels per core
NCOL = 64               # palette entries
KROWS = 21              # limb product rows: 6 per channel + 3 bias rows
PXT = 128               # pixels per matmul tile (stationary columns)
TPG = 16                # matmul tiles per group (2 PSUM banks)
GPX = PXT * TPG         # 2048 pixels per group
NTILES = HW // PXT      # 2048
GROUPS = NTILES // TPG  # 128
GPC = 16                # groups per out-DMA chunk
CHUNKS = GROUPS // GPC  # 8

_CACHE = {}


def _split3(x):
    """f32 -> three bf16 limbs (h, m, l) with h+m+l ~ x to ~2^-24 rel."""
    x = np.asarray(x, np.float32)
    h = x.astype(bf16)
    r = x - h.astype(np.float32)
    m = r.astype(bf16)
    r2 = r - m.astype(np.float32)
    l = r2.astype(bf16)
    return h, m, l


def _host_pixel_limbs(px):
    """px: (3, HW) f32 -> (GROUPS, KROWS, GPX) bf16 stationary slabs.

    Per channel the 6 product rows pair pixel limbs with palette limbs:
      (ph,wh) (ph,wm) (pm,wh) (pm,wm) (ph,wl) (pl,wh)
    so the pixel side is [ph, ph, pm, pm, ph, pl]; rows 18-20 are ones
    (paired with the 3 bias limbs).
    """
    rows = np.empty((KROWS, px.shape[1]), dtype=bf16)
    for c in range(3):
        h, m, l = _split3(px[c])
        rows[6 * c + 0] = h
        rows[6 * c + 1] = h
        rows[6 * c + 2] = m
        rows[6 * c + 3] = m
        rows[6 * c + 4] = h
        rows[6 * c + 5] = l
    rows[18:21] = np.ones((3, px.shape[1]), dtype=bf16)
    return np.ascontiguousarray(
        rows.reshape(KROWS, GROUPS, GPX).transpose(1, 0, 2)
    )


def _host_palette_rows(palette):
    """palette: (64, 3) f32 -> (KROWS, 64) bf16 moving-operand rows."""
    pal = np.asarray(palette, np.float64)
    rows = np.empty((KROWS, NCOL), dtype=bf16)
    for c in range(3):
        wh, wm, wl = _split3((2.0 * pal[:, c]).astype(np.float32))
        rows[6 * c + 0] = wh
        rows[6 * c + 1] = wm
        rows[6 * c + 2] = wh
        rows[6 * c + 3] = wm
        rows[6 * c + 4] = wl
        rows[6 * c + 5] = wh
    b64 = -(pal ** 2).sum(axis=1)
    bh = b64.astype(np.float32).astype(bf16)
    r = b64 - bh.astype(np.float64)
    bm = r.astype(np.float32).astype(bf16)
    r2 = r - bm.astype(np.float64)
    bl = r2.astype(np.float32).astype(bf16)
    rows[18], rows[19], rows[20] = bh, bm, bl
    return rows


def _host_negiota():
    """(128, TPG*NCOL) bf16: value 64-k at color slot k of every tile."""
    row = (NCOL - np.arange(NCOL, dtype=np.float32)).astype(bf16)
    return np.ascontiguousarray(
        np.broadcast_to(np.tile(row, TPG), (PXT, TPG * NCOL))
    )


def _build_body(nc, tc, ctx, aps, n_tiles):
    """Emit the per-core program. aps: dict name->AP for dram tensors."""
    import concourse.bass as bass
    import concourse.mybir as mybir

    n_groups = n_tiles // TPG
    f32 = mybir.dt.float32
    bft = mybir.dt.bfloat16

    consts = ctx.enter_context(tc.tile_pool(name="consts", bufs=1))
    limb_pool = ctx.enter_context(tc.tile_pool(name="limb", bufs=4))
    psum_pool = ctx.enter_context(tc.tile_pool(name="psum", bufs=4, space="PSUM"))
    cp_pool = ctx.enter_context(tc.tile_pool(name="cp", bufs=5))
    mx_pool = ctx.enter_context(tc.tile_pool(name="mx", bufs=4))
    eq_pool = ctx.enter_context(tc.tile_pool(name="eq", bufs=4))
    ni_pool = ctx.enter_context(tc.tile_pool(name="ni", bufs=4))
    stage_pool = ctx.enter_context(tc.tile_pool(name="stage", bufs=2))

    palT = consts.tile([KROWS, NCOL], bft)
    nc.sync.dma_start(palT[:], aps["palT"])
    negiota = consts.tile([PXT, TPG * NCOL], bft)
    nc.sync.dma_start(negiota[:], aps["negiota"])

    stages = {}
    cp_tiles = {}
    eq_tiles = {}
    ni_tiles = {}

    def emit_mult(h):
        """GpSimd payload multiply for group h, one iteration after E(h)
        was emitted — it then overlaps the next group's DVE work instead
        of serializing between DVE groups."""
        ni = ni_pool.tile([128, TPG * NCOL], bft, name="ni")
        nc.gpsimd.tensor_tensor(
            ni[:], eq_tiles.pop(h)[:], negiota[:], mybir.AluOpType.mult
        )
        ni_tiles[h] = ni

    def emit_consume(h):
        """R/E (DVE) + payload mult (GpSimd) for group h, one group after
        its producers — the ~1.1us Scalar->Vector semaphore/drain latency
        then hides behind the previous group's DVE work."""
        cp3 = cp_tiles.pop(h)[:].rearrange("p (t k) -> p t k", k=NCOL)
        mx = mx_pool.tile([128, TPG], f32, name="mx")
        nc.vector.tensor_reduce(
            mx[:], cp3, axis=mybir.AxisListType.X, op=mybir.AluOpType.max
        )
        eq = eq_pool.tile([128, TPG * NCOL], bft, name="eq")
        nc.vector.tensor_tensor(
            eq[:].rearrange("p (t k) -> p t k", k=NCOL),
            cp3,
            mx[:, :, None].to_broadcast([128, TPG, NCOL]),
            mybir.AluOpType.is_ge,
        )
        eq_tiles[h] = eq

    def emit_i2(h):
        """Final reduce for group h: ni -> win slice, + chunk DMA."""
        st = stages[h // GPC]
        j = h % GPC
        nc.vector.tensor_reduce(
            st[:, j * TPG:(j + 1) * TPG],
            ni_tiles.pop(h)[:].rearrange("p (t k) -> p t k", k=NCOL),
            axis=mybir.AxisListType.X,
            op=mybir.AluOpType.max,
        )
        if j == GPC - 1:
            nc.sync.dma_start(aps["win"][h // GPC], st[:])
            del stages[h // GPC]

    for g in range(n_groups):
        if g % GPC == 0:
            stages[g // GPC] = stage_pool.tile(
                [128, GPC * TPG], bft, name="stage"
            )
        lt = limb_pool.tile([KROWS, GPX], bft)
        nc.sync.dma_start(lt[:], aps["limbs"][g])

        ps = psum_pool.tile([128, TPG * NCOL], f32)
        for t in range(TPG):
            nc.tensor.matmul(
                ps[:, t * NCOL:(t + 1) * NCOL],
                lt[:, t * PXT:(t + 1) * PXT],
                palT[:],
                start=True,
                stop=True,
            )

        cp = cp_pool.tile([128, TPG * NCOL], f32, name="cp")
        nc.scalar.copy(cp[:], ps[:])
        cp_tiles[g] = cp

        if g > 1:
            emit_consume(g - 2)
        if g > 2:
            emit_mult(g - 3)
        if g > 3:
            emit_i2(g - 4)
    emit_consume(n_groups - 2)
    emit_consume(n_groups - 1)
    for h in range(n_groups - 3, n_groups):
        emit_mult(h)
    for h in range(n_groups - 4, n_groups):
        emit_i2(h)


def _build_nc(n_tiles):
    import concourse.bass as bass
    import concourse.mybir as mybir
    import concourse.tile as tile
    from concourse import bacc
    from contextlib import ExitStack

    n_groups = n_tiles // TPG
    nc = bacc.Bacc("TRN2", num_devices=8)
    aps = {
        "limbs": nc.dram_tensor(
            "limbs", (n_groups, KROWS, GPX), mybir.dt.bfloat16,
            kind="ExternalInput"
        ).ap(),
        "palT": nc.dram_tensor(
            "palT", (KROWS, NCOL), mybir.dt.bfloat16, kind="ExternalInput"
        ).ap(),
        "negiota": nc.dram_tensor(
            "negiota", (PXT, TPG * NCOL), mybir.dt.bfloat16,
            kind="ExternalInput"
        ).ap(),
        "win": nc.dram_tensor(
            "win", (n_groups // GPC, 128, GPC * TPG), mybir.dt.bfloat16,
            kind="ExternalOutput",
        ).ap(),
    }
    with tile.TileContext(nc) as tc:
        with ExitStack() as ctx:
            _build_body(nc, tc, ctx, aps, n_tiles)
    nc.compile()
    return nc


def _get_nc():
    if "nc" not in _CACHE:
        _CACHE["nc"] = _build_nc(NTILES)
    return _CACHE["nc"]


def _host_inputs(x, palette):
    """x: (B,3,H,W) f32, palette: (64,3) f32 -> list of 8 per-core in-maps."""
    palT = _host_palette_rows(palette)
    negiota = _host_negiota()
    in_maps = []
    for b in range(B):
        px = np.asarray(x[b], np.float32).reshape(3, HW)
        in_maps.append(
            {
                "limbs": _host_pixel_limbs(px),
                "palT": palT,
                "negiota": negiota,
            }
        )
    return in_maps


def _host_finish(win_list, palette):
    """win: per-core (GROUPS, 128, TPG) bf16 -> (B,3,H,W) f32 output."""
    pal = np.asarray(palette, np.float32)
    out = np.empty((B, 3, H, W), np.float32)
    for b, wv in enumerate(win_list):
        w = np.asarray(wv).astype(np.float32)
        idx = (NCOL - w).astype(np.int64)               # (chunk, p, j*TPG+t)
        np.clip(idx, 0, NCOL - 1, out=idx)
        idx = idx.reshape(CHUNKS, 128, GPC, TPG)        # (c, p, j, t)
        idx = idx.transpose(0, 2, 3, 1).reshape(HW)     # (c, j, t, p) -> px
        out[b] = pal[idx].T.reshape(3, H, W)
    return out


def kernel(x, palette):
    from concourse.bass_utils import run_bass_kernel_spmd

    nc = _get_nc()
    in_maps = _host_inputs(x, palette)
    res = run_bass_kernel_spmd(nc, in_maps, core_ids=list(range(8)))
    win = [res.results[i]["win"] for i in range(B)]
    _CACHE["last_results"] = res
    return _host_finish(win, palette).astype(np.float32)


# revision 20
# speedup vs baseline: 9.3552x; 1.3997x over previous
"""nn_ColorReducer — Trainium2 Bass kernel (8-core data-parallel).

Algorithm (exact nearest-palette-color via argmax-index extraction):
  score_k(p) = 2 p.c_k - ||c_k||^2  (maximizing k <=> nearest color)
  Scores on TensorE with 128-pixel tiles stationary (21 bf16 limb rows for
  ~2^-24 accuracy) and the palette moving, landing in PSUM as
  (128 px, 64 colors) — reducible along the free dim. Index extraction:
    mx  = reduce_max(scores)            [DVE]
    eq  = (scores >= mx) in bf16        [GpSimd]
    ni  = eq * (64 - k)                 [DVE, materialized reverse-iota]
    win = reduce_max(ni)                [DVE]  -> k* = 64 - win
  (reverse iota makes ties pick the smallest k, matching jnp.argmin).
  Only the 2-byte win indices are shipped out; the host gathers
  palette[k*] and reshapes.

Sharding: batch dim (8 images) across the 8 NeuronCores; palette replicated.
"""

import numpy as np
import ml_dtypes

bf16 = ml_dtypes.bfloat16

B, C, H, W = 8, 3, 512, 512
HW = H * W              # 262144 pixels per core
NCOL = 64               # palette entries
KROWS = 21              # limb product rows: 6 per channel + 3 bias rows
PXT = 128               # pixels per matmul tile (stationary columns)
TPG = 16                # matmul tiles per group (2 PSUM banks)
GPX = PXT * TPG         # 2048 pixels per group
NTILES = HW // PXT      # 2048
GROUPS = NTILES // TPG  # 128
GPC = 16                # groups per out-DMA chunk
CHUNKS = GROUPS // GPC  # 8

_CACHE = {}


def _split3(x):
    """f32 -> three bf16 limbs (h, m, l) with h+m+l ~ x to ~2^-24 rel."""
    x = np.asarray(x, np.float32)
    h = x.astype(bf16)
    r = x - h.astype(np.float32)
    m = r.astype(bf16)
    r2 = r - m.astype(np.float32)
    l = r2.astype(bf16)
    return h, m, l


def _host_pixel_limbs(px):
    """px: (3, HW) f32 -> (GROUPS, KROWS, GPX) bf16 stationary slabs.

    Per channel the 6 product rows pair pixel limbs with palette limbs:
      (ph,wh) (ph,wm) (pm,wh) (pm,wm) (ph,wl) (pl,wh)
    so the pixel side is [ph, ph, pm, pm, ph, pl]; rows 18-20 are ones
    (paired with the 3 bias limbs).
    """
    rows = np.empty((KROWS, px.shape[1]), dtype=bf16)
    for c in range(3):
        h, m, l = _split3(px[c])
        rows[6 * c + 0] = h
        rows[6 * c + 1] = h
        rows[6 * c + 2] = m
        rows[6 * c + 3] = m
        rows[6 * c + 4] = h
        rows[6 * c + 5] = l
    rows[18:21] = np.ones((3, px.shape[1]), dtype=bf16)
    return np.ascontiguousarray(
        rows.reshape(KROWS, GROUPS, GPX).transpose(1, 0, 2)
    )


def _host_palette_rows(palette):
    """palette: (64, 3) f32 -> (KROWS, 64) bf16 moving-operand rows."""
    pal = np.asarray(palette, np.float64)
    rows = np.empty((KROWS, NCOL), dtype=bf16)
    for c in range(3):
        wh, wm, wl = _split3((2.0 * pal[:, c]).astype(np.float32))
        rows[6 * c + 0] = wh
        rows[6 * c + 1] = wm
        rows[6 * c + 2] = wh
        rows[6 * c + 3] = wm
        rows[6 * c + 4] = wl
        rows[6 * c + 5] = wh
    b64 = -(pal ** 2).sum(axis=1)
    bh = b64.astype(np.float32).astype(bf16)
    r = b64 - bh.astype(np.float64)
    bm = r.astype(np.float32).astype(bf16)
    r2 = r - bm.astype(np.float64)
    bl = r2.astype(np.float32).astype(bf16)
    rows[18], rows[19], rows[20] = bh, bm, bl
    return rows


def _host_negiota():
    """(128, TPG*32) bf16: value 32-j at pair slot j of every tile."""
    row = (32 - np.arange(32, dtype=np.float32)).astype(bf16)
    return np.ascontiguousarray(
        np.broadcast_to(np.tile(row, TPG), (PXT, TPG * 32))
    )


def _build_body(nc, tc, ctx, aps, n_tiles):
    """Emit the per-core program. aps: dict name->AP for dram tensors."""
    import concourse.bass as bass
    import concourse.mybir as mybir

    n_groups = n_tiles // TPG
    f32 = mybir.dt.float32
    bft = mybir.dt.bfloat16

    consts = ctx.enter_context(tc.tile_pool(name="consts", bufs=1))
    limb_pool = ctx.enter_context(tc.tile_pool(name="limb", bufs=4))
    psum_pool = ctx.enter_context(tc.tile_pool(name="psum", bufs=4, space="PSUM"))
    cp_pool = ctx.enter_context(tc.tile_pool(name="cp", bufs=5))
    mx_pool = ctx.enter_context(tc.tile_pool(name="mx", bufs=4))
    eq_pool = ctx.enter_context(tc.tile_pool(name="eq", bufs=4))
    ni_pool = ctx.enter_context(tc.tile_pool(name="ni", bufs=4))
    ch_pool = ctx.enter_context(tc.tile_pool(name="ch", bufs=4))
    stage_pool = ctx.enter_context(tc.tile_pool(name="stage", bufs=2))

    palT = consts.tile([KROWS, NCOL], bft)
    nc.sync.dma_start(palT[:], aps["palT"])
    negiota = consts.tile([PXT, TPG * 32], bft)
    nc.sync.dma_start(negiota[:], aps["negiota"])

    stages = {}
    cp_tiles = {}
    ch_tiles = {}
    eq_tiles = {}
    ni_tiles = {}

    def emit_halfmax(h):
        """GpSimd pairwise max of colors k vs k+32: 64 scores -> 32 pair
        maxima, halving every downstream DVE pass. The host resolves the
        final bit exactly from x and the palette."""
        cp3 = cp_tiles.pop(h)[:].rearrange("p (t k) -> p t k", k=NCOL)
        ch = ch_pool.tile([128, TPG * 32], f32, name="ch")
        nc.vector.tensor_tensor(
            ch[:].rearrange("p (t j) -> p t j", j=32),
            cp3[:, :, 0:32],
            cp3[:, :, 32:64],
            mybir.AluOpType.max,
        )
        ch_tiles[h] = ch

    def emit_mult(h):
        """GpSimd payload multiply for group h, one iteration after E(h)
        was emitted — it then overlaps the next group's DVE work instead
        of serializing between DVE groups."""
        ni = ni_pool.tile([128, TPG * 32], bft, name="ni")
        nc.gpsimd.tensor_tensor(
            ni[:], eq_tiles.pop(h)[:], negiota[:], mybir.AluOpType.mult
        )
        ni_tiles[h] = ni

    def emit_consume(h):
        """R/E (DVE) + payload mult (GpSimd) for group h, one group after
        its producers — the ~1.1us Scalar->Vector semaphore/drain latency
        then hides behind the previous group's DVE work."""
        ch3 = ch_tiles.pop(h)[:].rearrange("p (t j) -> p t j", j=32)
        mx = mx_pool.tile([128, TPG], f32, name="mx")
        nc.vector.tensor_reduce(
            mx[:], ch3, axis=mybir.AxisListType.X, op=mybir.AluOpType.max
        )
        eq = eq_pool.tile([128, TPG * 32], bft, name="eq")
        nc.vector.tensor_tensor(
            eq[:].rearrange("p (t j) -> p t j", j=32),
            ch3,
            mx[:, :, None].to_broadcast([128, TPG, 32]),
            mybir.AluOpType.is_ge,
        )
        eq_tiles[h] = eq

    def emit_i2(h):
        """Final reduce for group h: ni -> win slice, + chunk DMA."""
        st = stages[h // GPC]
        j = h % GPC
        nc.vector.tensor_reduce(
            st[:, j * TPG:(j + 1) * TPG],
            ni_tiles.pop(h)[:].rearrange("p (t j) -> p t j", j=32),
            axis=mybir.AxisListType.X,
            op=mybir.AluOpType.max,
        )
        if j == GPC - 1:
            nc.sync.dma_start(aps["win"][h // GPC], st[:])
            del stages[h // GPC]

    for g in range(n_groups):
        if g % GPC == 0:
            stages[g // GPC] = stage_pool.tile(
                [128, GPC * TPG], bft, name="stage"
            )
        lt = limb_pool.tile([KROWS, GPX], bft)
        nc.sync.dma_start(lt[:], aps["limbs"][g])

        ps = psum_pool.tile([128, TPG * NCOL], f32)
        for t in range(TPG):
            nc.tensor.matmul(
                ps[:, t * NCOL:(t + 1) * NCOL],
                lt[:, t * PXT:(t + 1) * PXT],
                palT[:],
                start=True,
                stop=True,
            )

        cp = cp_pool.tile([128, TPG * NCOL], f32, name="cp")
        nc.scalar.copy(cp[:], ps[:])
        cp_tiles[g] = cp

        if g > 0:
            emit_halfmax(g - 1)
        if g > 1:
            emit_consume(g - 2)
        if g > 2:
            emit_mult(g - 3)
        if g > 3:
            emit_i2(g - 4)
    emit_halfmax(n_groups - 1)
    emit_consume(n_groups - 2)
    emit_consume(n_groups - 1)
    for h in range(n_groups - 3, n_groups):
        emit_mult(h)
    for h in range(n_groups - 4, n_groups):
        emit_i2(h)


def _build_nc(n_tiles):
    import concourse.bass as bass
    import concourse.mybir as mybir
    import concourse.tile as tile
    from concourse import bacc
    from contextlib import ExitStack

    n_groups = n_tiles // TPG
    nc = bacc.Bacc("TRN2", num_devices=8)
    aps = {
        "limbs": nc.dram_tensor(
            "limbs", (n_groups, KROWS, GPX), mybir.dt.bfloat16,
            kind="ExternalInput"
        ).ap(),
        "palT": nc.dram_tensor(
            "palT", (KROWS, NCOL), mybir.dt.bfloat16, kind="ExternalInput"
        ).ap(),
        "negiota": nc.dram_tensor(
            "negiota", (PXT, TPG * 32), mybir.dt.bfloat16,
            kind="ExternalInput"
        ).ap(),
        "win": nc.dram_tensor(
            "win", (n_groups // GPC, 128, GPC * TPG), mybir.dt.bfloat16,
            kind="ExternalOutput",
        ).ap(),
    }
    with tile.TileContext(nc) as tc:
        with ExitStack() as ctx:
            _build_body(nc, tc, ctx, aps, n_tiles)
    nc.compile()
    return nc


def _get_nc():
    if "nc" not in _CACHE:
        _CACHE["nc"] = _build_nc(NTILES)
    return _CACHE["nc"]


def _host_inputs(x, palette):
    """x: (B,3,H,W) f32, palette: (64,3) f32 -> list of 8 per-core in-maps."""
    palT = _host_palette_rows(palette)
    negiota = _host_negiota()
    in_maps = []
    for b in range(B):
        px = np.asarray(x[b], np.float32).reshape(3, HW)
        in_maps.append(
            {
                "limbs": _host_pixel_limbs(px),
                "palT": palT,
                "negiota": negiota,
            }
        )
    return in_maps


def _host_finish(win_list, palette, x):
    """win: per-core winning PAIR j (k = j or j+32); host picks the member
    exactly from x and the palette."""
    pal = np.asarray(palette, np.float32)
    out = np.empty((B, 3, H, W), np.float32)
    for b, wv in enumerate(win_list):
        w = np.asarray(wv).astype(np.float32)
        j = (32 - w).astype(np.int64)                   # (chunk, p, j*TPG+t)
        np.clip(j, 0, 31, out=j)
        j = j.reshape(CHUNKS, 128, GPC, TPG)            # (c, p, g, t)
        j = j.transpose(0, 2, 3, 1).reshape(HW)         # (c, g, t, p) -> px
        px = np.asarray(x[b], np.float32).reshape(3, HW).T
        c0 = pal[j]
        c1 = pal[j + 32]
        d0 = ((px - c0) ** 2).sum(1)
        d1 = ((px - c1) ** 2).sum(1)
        idx = np.where(d1 < d0, j + 32, j)
        out[b] = pal[idx].T.reshape(3, H, W)
    return out


def kernel(x, palette):
    from concourse.bass_utils import run_bass_kernel_spmd

    nc = _get_nc()
    in_maps = _host_inputs(x, palette)
    res = run_bass_kernel_spmd(nc, in_maps, core_ids=list(range(8)))
    win = [res.results[i]["win"] for i in range(B)]
    _CACHE["last_results"] = res
    return _host_finish(win, palette, x).astype(np.float32)
